# revision 1
# baseline (speedup 1.0000x reference)
"""DiagSSMBlock Trainium2 kernel.

h_t = sum_{k=0..t} a^k * (B^T x_{t-k})  ==  h_t = a * h_{t-1} + s_t, s = B^T x^T.

Strategy: shard T across the 8 cores (1024 steps each + 32-step halo; |a| <=
sqrt(2/1024) ~ 0.044 so a^32 < 1e-43 == 0 in fp32, making slabs exactly
independent).  Host passes x pre-transposed ([H, T_slab]) so the tensor engine
can contract over H with no on-chip transposes; the scan output is returned
channel-major [H, T_slab] and transposed back on host.

Per core: DMA B + xT slab -> 8x K-chunked fp32r matmul accumulation into PSUM
(3 chunks of 352 time-cols) -> tensor_tensor_scan (the SSM recurrence) per
128-channel group -> DMA out.  Dummy warm-up matmuls run during the input DMA
ramp so the PE HAM clock-gate reaches 2.4 GHz before the real matmuls start;
DMA issue is spread across the two HWDGE engines (sync + scalar).

Scheduling note: the a-broadcast tiles are built with gpsimd tensor_scalar,
which is slow (~5us each) -- measured FASTER end-to-end than building them on
DVE, because the slow drip of a_bc completions paces scan start (and hence
psum-slot recycling and output stores) to roughly match DMA supply, keeping
the DVE out of the PE's way during the input-bound phase.
"""

import sys

if "/opt/trn_rl_repo" not in sys.path:
    sys.path.insert(0, "/opt/trn_rl_repo")

import numpy as np

T, H = 8192, 1024
NC = 8
P = 128
T_LOC = T // NC            # 1024 output timesteps per core
HALO = 32                  # scan warmup; a^32 == 0 in fp32
W = T_LOC + HALO           # 1056
CH = 352                   # psum chunk width (3 chunks of 352 = 1056)
NCHUNK = W // CH
KQ = H // P                # 8 contraction chunks
G = H // P                 # 8 channel groups
N_WARM = 12                # dummy matmuls to lift the HAM clock gate

MM_DTYPE = "float32r"      # matmul operand dtype: "float32" (4 cyc/row) or
                           # "float32r" (1 cyc/row at N>=256)

_state = {}


def _build_nc():
    import concourse.tile as tile
    from concourse import bacc, mybir

    mm_dt = getattr(mybir.dt, MM_DTYPE)
    f32 = mybir.dt.float32

    nc = bacc.Bacc("TRN2", target_bir_lowering=False, debug=False, num_devices=NC)
    xt_e = nc.dram_tensor("xt", [H, W], mm_dt, kind="ExternalInput").ap()
    b_e = nc.dram_tensor("b", [H, H], mm_dt, kind="ExternalInput").ap()
    av_e = nc.dram_tensor("av", [P, G], f32, kind="ExternalInput").ap()
    out_e = nc.dram_tensor("out", [H, T_LOC], f32, kind="ExternalOutput").ap()
    flush_e = nc.dram_tensor("warm_flush", [P, 1], f32).ap()

    with tile.TileContext(nc) as tc:
        with (
            tc.tile_pool(name="consts", bufs=1) as consts,
            tc.tile_pool(name="bpool", bufs=1) as bpool,
            tc.tile_pool(name="xpool", bufs=1) as xpool,
            tc.tile_pool(name="hpool", bufs=1) as hpool,
            tc.tile_pool(name="pspool", bufs=6, space="PSUM") as pspool,
            tc.tile_pool(name="warmps", bufs=1, space="PSUM") as warmps,
        ):
            # PE warm-up: dummy fp32 matmuls on a zeroed scratch tile, gated
            # only on a gpsimd memset, so the HAM clock-gate lifts to 2.4 GHz
            # during the input-DMA ramp.
            warm_sb = consts.tile([P, P], f32, tag="warm")
            nc.gpsimd.memset(warm_sb[:], 0.0)
            wps = warmps.tile([P, P], f32)
            for i in range(N_WARM):
                nc.tensor.matmul(
                    wps[:],
                    warm_sb[:],
                    warm_sb[:],
                    start=(i == 0),
                    stop=(i == N_WARM - 1),
                )
            flush_sb = consts.tile([P, 1], f32, tag="flush")
            nc.vector.tensor_copy(flush_sb[:], wps[:, 0:1])
            nc.gpsimd.dma_start(flush_e[:], flush_sb[:])

            # a broadcast tiles (built on gpsimd; see module docstring)
            av_sb = consts.tile([P, G], f32, tag="av")
            nc.sync.dma_start(av_sb[:], av_e[:])
            a_bc = []
            for g in range(G):
                t = consts.tile([P, CH], f32, tag=f"abc{g}")
                nc.gpsimd.memset(t[:], 1.0)
                nc.gpsimd.tensor_scalar_mul(t[:], t[:], av_sb[:, g : g + 1])
                a_bc.append(t)

            # Input loads.  sync carries xt chunks 0 and 2; scalar carries the
            # group-0 b tiles (fine-grained for a fast start), xt chunk 1, the
            # rest of b, and the output stores.
            b_g0 = []
            for kq in range(KQ):
                bt = bpool.tile([P, P], mm_dt, tag=f"bg0_{kq}")
                nc.scalar.dma_start(
                    bt[:], b_e[kq * P : (kq + 1) * P, 0:P]
                )
                b_g0.append(bt)
            xt_sb = [[None] * NCHUNK for _ in range(KQ)]
            for ni in range(NCHUNK):
                eng = nc.scalar if ni == 1 else nc.sync
                n0 = ni * CH
                for kq in range(KQ):
                    xtile = xpool.tile([P, CH], mm_dt, tag=f"x{kq}_{ni}")
                    eng.dma_start(
                        xtile[:], xt_e[kq * P : (kq + 1) * P, n0 : n0 + CH]
                    )
                    xt_sb[kq][ni] = xtile
            b_rest = []
            for kq in range(KQ):
                bt = bpool.tile([P, H - P], mm_dt, tag=f"br_{kq}")
                nc.scalar.dma_start(bt[:], b_e[kq * P : (kq + 1) * P, P:H])
                b_rest.append(bt)

            def b_slice(kq, g):
                if g == 0:
                    return b_g0[kq][:]
                return b_rest[kq][:, (g - 1) * P : g * P]

            for g in range(G):
                h_t = hpool.tile([P, W], f32, tag=f"h{g}")
                for ni in range(NCHUNK):
                    n0 = ni * CH
                    ps = pspool.tile([P, CH], f32)
                    for kq in range(KQ):
                        nc.tensor.matmul(
                            ps[:],
                            b_slice(kq, g),
                            xt_sb[kq][ni][:],
                            start=(kq == 0),
                            stop=(kq == KQ - 1),
                        )
                    init = 0.0 if ni == 0 else h_t[:, n0 - 1 : n0]
                    nc.vector.tensor_tensor_scan(
                        h_t[:, n0 : n0 + CH],
                        a_bc[g][:],
                        ps[:],
                        init,
                        op0=mybir.AluOpType.mult,
                        op1=mybir.AluOpType.add,
                    )
                    if g < 3:
                        # keep-warm fillers: occupy the PE during input-DMA
                        # stalls of the early phase so HAM stays at 2.4 GHz
                        for i in range(2):
                            nc.tensor.matmul(
                                wps[:],
                                warm_sb[:],
                                warm_sb[:],
                                start=(i == 0),
                                stop=(i == 1),
                            )
                nc.scalar.dma_start(out_e[g * P : (g + 1) * P, :], h_t[:, HALO:W])

    nc.compile()
    return nc


def _get_nc():
    if "nc" not in _state:
        _state["nc"] = _build_nc()
    return _state["nc"]


def _shard_inputs(x_seq, a_diag, b_mat):
    x = np.asarray(x_seq, dtype=np.float32)
    a = np.asarray(a_diag, dtype=np.float32)
    b = np.ascontiguousarray(np.asarray(b_mat, dtype=np.float32))
    x_pad = np.concatenate([np.zeros((HALO, H), np.float32), x], axis=0)
    xT = np.ascontiguousarray(x_pad.T)  # [H, T + HALO]
    av = np.ascontiguousarray(a.reshape(G, P).T)  # [P, G]
    in_maps = []
    for i in range(NC):
        in_maps.append(
            {
                "xt": np.ascontiguousarray(xT[:, i * T_LOC : i * T_LOC + W]),
                "b": b,
                "av": av,
            }
        )
    return in_maps


def kernel(x_seq, a_diag, b_mat):
    from concourse.bass_utils import run_bass_kernel_spmd

    nc = _get_nc()
    in_maps = _shard_inputs(x_seq, a_diag, b_mat)
    res = run_bass_kernel_spmd(nc, in_maps, list(range(NC)))
    _state["last_result"] = res
    out = np.concatenate(
        [np.asarray(res.results[i]["out"]).T for i in range(NC)], axis=0
    )
    return out



# revision 4
# speedup vs baseline: 1.0301x; 1.0301x over previous
"""DiagSSMBlock Trainium2 kernel.

h_t = sum_{k=0..t} a^k * (B^T x_{t-k})  ==  h_t = a * h_{t-1} + s_t, s = B^T x^T.

Strategy: shard T across the 8 cores (1024 steps each + 8-step halo; |a| <=
sqrt(2/1024) ~ 0.044 so a^8 ~ 1.5e-11 — far below the 2e-2 gate and below
fp32 noise, making slabs effectively independent).  Host passes x
pre-transposed ([H, T_slab]) and pre-cast to bf16 along with B; bf16 keeps
the PE at 1 cycle/row (same as fp32r) while HALVING the HBM traffic
(4.2 MB in + 2 MB out per core vs 12.3 MB fp32), so DMA fully hides under
the ~28 us matmul stream.

Per core, kq-MAJOR accumulation: sweep a chunk of 344 time-cols through all
8 PSUM banks (one per 128-channel output group), contracting one 128-row K
chunk per pass.  A pass needs only x-chunk[kq] (88 KB) + b-row[kq] (262 KB),
so real matmuls start after ~350 KB of DMA instead of after the whole 4.2 MB
K-sweep worth of input.  DVE runs the SSM recurrence (tensor_tensor_scan)
per group right behind each sweep's last accumulation, writing bf16 h tiles
that are immediately stored per (group, chunk) — no serial scan/store tail.
a-broadcast tiles are built on DVE early (gpsimd is ~13x slower at this and
paced the old kernel's scans into a 12 us tail).

Warm-up matmuls run during the input DMA ramp so the PE HAM clock-gate
reaches 2.4 GHz quickly; all input loads are issued on sync in consumption
order, output stores on scalar.
"""

import sys

if "/opt/trn_rl_repo" not in sys.path:
    sys.path.insert(0, "/opt/trn_rl_repo")

import ml_dtypes
import numpy as np

T, H = 8192, 1024
NC = 8
P = 128
T_LOC = T // NC            # 1024 output timesteps per core
HALO = 8                   # scan warmup; a^8 ~ 1.5e-11
W = T_LOC + HALO           # 1032
CH = 344                   # psum chunk width (3 chunks of 344 = 1032)
NCHUNK = W // CH           # 3
KQ = H // P                # 8 contraction chunks
G = H // P                 # 8 output-channel groups
N_WARM = 12                # dummy matmuls to lift the HAM clock gate

_state = {}


def _build_nc():
    import concourse.tile as tile
    from concourse import bacc, mybir

    bf16 = mybir.dt.bfloat16
    f32 = mybir.dt.float32

    nc = bacc.Bacc("TRN2", target_bir_lowering=False, debug=False, num_devices=NC)
    xt_e = nc.dram_tensor("xt", [H, W], bf16, kind="ExternalInput").ap()
    b_e = nc.dram_tensor("b", [H, H], bf16, kind="ExternalInput").ap()
    av_e = nc.dram_tensor("av", [P, G], f32, kind="ExternalInput").ap()
    out_e = nc.dram_tensor("out", [H, T_LOC], bf16, kind="ExternalOutput").ap()
    flush_e = nc.dram_tensor("warm_flush", [P, 1], f32).ap()

    with tile.TileContext(nc) as tc:
        with (
            tc.tile_pool(name="consts", bufs=1) as consts,
            tc.tile_pool(name="bpool", bufs=1) as bpool,
            tc.tile_pool(name="xpool", bufs=1) as xpool,
            tc.tile_pool(name="hpool", bufs=1) as hpool,
            tc.tile_pool(name="pspool", bufs=8, space="PSUM") as pspool,
        ):
            # PE warm-up: dummy bf16 matmuls on a zeroed scratch tile, gated
            # only on a gpsimd memset, so the HAM clock-gate lifts to 2.4 GHz
            # during the input-DMA ramp.
            warm_sb = consts.tile([P, P], bf16, tag="warm")
            nc.gpsimd.memset(warm_sb[:], 0.0)
            wps = pspool.tile([P, P], f32, tag="ps", name="wps")
            for i in range(N_WARM):
                nc.tensor.matmul(
                    wps[:],
                    warm_sb[:],
                    warm_sb[:],
                    start=(i == 0),
                    stop=(i == N_WARM - 1),
                )
            flush_sb = consts.tile([P, 1], f32, tag="flush")
            nc.vector.tensor_copy(flush_sb[:], wps[:, 0:1])
            nc.gpsimd.dma_start(flush_e[:], flush_sb[:])

            # a-broadcast tiles, built on DVE (fast; ready well before the
            # first scan needs them).
            av_sb = consts.tile([P, G], f32, tag="av")
            nc.sync.dma_start(av_sb[:], av_e[:])
            ones = consts.tile([P, CH], f32, tag="ones")
            nc.vector.memset(ones[:], 1.0)
            a_bc = []
            for g in range(G):
                t = consts.tile([P, CH], f32, tag=f"abc{g}", name=f"abc{g}")
                nc.vector.tensor_scalar_mul(t[:], ones[:], av_sb[:, g : g + 1])
                a_bc.append(t)

            # Input loads, all on sync's HWDGE ring, in exact consumption
            # order: (x chunk-0, b row) per kq unlock the kq-major passes one
            # by one; x chunks 1-2 follow and arrive long before sweep 1.
            x0, b_sb = [], []
            for kq in range(KQ):
                xt0 = xpool.tile([P, CH], bf16, tag=f"x0_{kq}", name=f"x0_{kq}")
                nc.sync.dma_start(xt0[:], xt_e[kq * P : (kq + 1) * P, 0:CH])
                x0.append(xt0)
                bt = bpool.tile([P, H], bf16, tag=f"b_{kq}", name=f"b_{kq}")
                nc.sync.dma_start(bt[:], b_e[kq * P : (kq + 1) * P, :])
                b_sb.append(bt)
            x12 = []
            for kq in range(KQ):
                xt12 = xpool.tile(
                    [P, W - CH], bf16, tag=f"x12_{kq}", name=f"x12_{kq}"
                )
                nc.sync.dma_start(xt12[:], xt_e[kq * P : (kq + 1) * P, CH:W])
                x12.append(xt12)

            def x_chunk(kq, ni):
                if ni == 0:
                    return x0[kq][:]
                return x12[kq][:, (ni - 1) * CH : ni * CH]

            h_t = [
                hpool.tile([P, W], bf16, tag=f"h{g}", name=f"h{g}")
                for g in range(G)
            ]

            for ni in range(NCHUNK):
                ps = [
                    pspool.tile([P, CH], f32, tag="ps", name=f"ps{ni}_{g}")
                    for g in range(G)
                ]
                for kq in range(KQ):
                    for g in range(G):
                        nc.tensor.matmul(
                            ps[g][:],
                            b_sb[kq][:, g * P : (g + 1) * P],
                            x_chunk(kq, ni),
                            start=(kq == 0),
                            stop=(kq == KQ - 1),
                        )
                n0 = ni * CH
                for g in range(G):
                    init = 0.0 if ni == 0 else h_t[g][:, n0 - 1 : n0]
                    nc.vector.tensor_tensor_scan(
                        h_t[g][:, n0 : n0 + CH],
                        a_bc[g][:],
                        ps[g][:],
                        init,
                        op0=mybir.AluOpType.mult,
                        op1=mybir.AluOpType.add,
                    )
                    lo = HALO if ni == 0 else 0
                    nc.scalar.dma_start(
                        out_e[g * P : (g + 1) * P, n0 + lo - HALO : n0 + CH - HALO],
                        h_t[g][:, n0 + lo : n0 + CH],
                    )

    nc.compile()
    return nc


def _get_nc():
    if "nc" not in _state:
        _state["nc"] = _build_nc()
    return _state["nc"]


def _shard_inputs(x_seq, a_diag, b_mat):
    x = np.asarray(x_seq, dtype=np.float32)
    a = np.asarray(a_diag, dtype=np.float32)
    b = np.asarray(b_mat, dtype=np.float32)
    bq = np.ascontiguousarray(b.astype(ml_dtypes.bfloat16))
    x_pad = np.concatenate([np.zeros((HALO, H), np.float32), x], axis=0)
    xT = np.ascontiguousarray(x_pad.T).astype(ml_dtypes.bfloat16)  # [H, T+HALO]
    av = np.ascontiguousarray(a.reshape(G, P).T)  # [P, G]
    in_maps = []
    for i in range(NC):
        in_maps.append(
            {
                "xt": np.ascontiguousarray(xT[:, i * T_LOC : i * T_LOC + W]),
                "b": bq,
                "av": av,
            }
        )
    return in_maps


def kernel(x_seq, a_diag, b_mat):
    from concourse.bass_utils import run_bass_kernel_spmd

    nc = _get_nc()
    in_maps = _shard_inputs(x_seq, a_diag, b_mat)
    res = run_bass_kernel_spmd(nc, in_maps, list(range(NC)))
    _state["last_result"] = res
    out = np.concatenate(
        [
            np.asarray(res.results[i]["out"]).astype(np.float32).T
            for i in range(NC)
        ],
        axis=0,
    )
    return out


# revision 5
# speedup vs baseline: 1.2192x; 1.1836x over previous
"""DiagSSMBlock Trainium2 kernel.

h_t = sum_{k=0..t} a^k * (B^T x_{t-k})  ==  h_t = a * h_{t-1} + s_t, s = B^T x^T.

Strategy: shard T across the 8 cores (1024 steps each + 8-step halo; |a| <=
sqrt(2/1024) ~ 0.044 so a^8 ~ 1.5e-11 — invisible at fp32, making slabs
independent).  Host passes x pre-transposed ([H, T_slab]) and pre-cast to
bf16 along with B; bf16 keeps the PE at 1 cycle/row (same rate as fp32r)
while halving HBM traffic (4.2 MB in + 2 MB out per core vs 12.3 MB fp32).

Measured constraints this layout is built around (from ntff traces):
  - each dma_start occupies the issuing engine's NX ~0.6 us, so loads are
    split across BOTH HWDGE rings (sync + scalar) in consumption order;
  - a ~7 us per-engine framework preamble precedes everything; warm-up
    matmuls bridge the gap from preamble end to first-data arrival so the
    PE HAM clock-gate lifts to 2.4 GHz right as real matmuls start;
  - chunk ni=0 runs kq-MAJOR across all 8 PSUM banks (one per 128-channel
    group): a pass consumes one (x-chunk, b-row) pair = 350 KB, matching
    the DMA arrival cadence, so compute starts ~2 us after the preamble
    instead of waiting for the full 2.8 MB K-sweep;
  - chunks 1-2 run g-major (8 accumulations then scan per group) so the
    DVE scan of group g frees its PSUM bank just ahead of the next chunk's
    use — no 8-scan serial barrier between chunks;
  - the SSM recurrence itself is DVE tensor_tensor_scan (state fp32, bf16
    out), with the per-partition `a` supplied as a stride-0 broadcast AP.
"""

import sys

if "/opt/trn_rl_repo" not in sys.path:
    sys.path.insert(0, "/opt/trn_rl_repo")

import ml_dtypes
import numpy as np

T, H = 8192, 1024
NC = 8
P = 128
T_LOC = T // NC            # 1024 output timesteps per core
HALO = 8                   # scan warmup; a^8 ~ 1.5e-11
W = T_LOC + HALO           # 1032
CH = 344                   # psum chunk width (3 chunks of 344 = 1032)
NCHUNK = W // CH           # 3
KQ = H // P                # 8 contraction chunks
G = H // P                 # 8 output-channel groups
N_WARM = 20                # dummy matmuls to lift the HAM clock gate
USE_BCAST = True           # a as stride-0 broadcast AP (else DVE-built tiles)

_state = {}


def _build_nc():
    import concourse.tile as tile
    from concourse import bacc, mybir

    bf16 = mybir.dt.bfloat16
    f32 = mybir.dt.float32

    nc = bacc.Bacc("TRN2", target_bir_lowering=False, debug=False, num_devices=NC)
    xt_e = nc.dram_tensor("xt", [H, W], bf16, kind="ExternalInput").ap()
    b_e = nc.dram_tensor("b", [H, H], bf16, kind="ExternalInput").ap()
    av_e = nc.dram_tensor("av", [P, G], f32, kind="ExternalInput").ap()
    out_e = nc.dram_tensor("out", [H, T_LOC], bf16, kind="ExternalOutput").ap()
    flush_e = nc.dram_tensor("warm_flush", [P, 1], f32).ap()

    with tile.TileContext(nc) as tc:
        with (
            tc.tile_pool(name="consts", bufs=1) as consts,
            tc.tile_pool(name="bpool", bufs=1) as bpool,
            tc.tile_pool(name="xpool", bufs=1) as xpool,
            tc.tile_pool(name="hpool", bufs=1) as hpool,
            tc.tile_pool(name="pspool", bufs=8, space="PSUM") as pspool,
        ):
            # PE warm-up: bridge preamble-end -> first-data with dummy MMs.
            warm_sb = consts.tile([P, P], bf16, tag="warm")
            nc.gpsimd.memset(warm_sb[:], 0.0)
            wps = pspool.tile([P, P], f32, tag="ps", name="wps")
            for i in range(N_WARM):
                nc.tensor.matmul(
                    wps[:],
                    warm_sb[:],
                    warm_sb[:],
                    start=(i == 0),
                    stop=(i == N_WARM - 1),
                )
            flush_sb = consts.tile([P, 1], f32, tag="flush")
            nc.vector.tensor_copy(flush_sb[:], wps[:, 0:1])
            nc.gpsimd.dma_start(flush_e[:], flush_sb[:])

            # a values (tiny, on gpsimd's SWDGE ring to keep HWDGE rings free)
            av_sb = consts.tile([P, G], f32, tag="av")
            nc.gpsimd.dma_start(av_sb[:], av_e[:])
            if USE_BCAST:
                a_op = [av_sb[:, g : g + 1].broadcast_to([P, CH]) for g in range(G)]
            else:
                ones = consts.tile([P, CH], f32, tag="ones")
                nc.vector.memset(ones[:], 1.0)
                a_op = []
                for g in range(G):
                    t = consts.tile([P, CH], f32, tag=f"abc{g}", name=f"abc{g}")
                    nc.vector.tensor_scalar_mul(t[:], ones[:], av_sb[:, g : g + 1])
                    a_op.append(t[:])

            # Input loads interleaved across the two HWDGE rings in
            # consumption order; stores go on scalar after its loads.
            x0, b_sb, x12 = [], [], []
            for kq in range(KQ):
                eng = nc.sync if kq % 2 == 0 else nc.scalar
                xt0 = xpool.tile([P, CH], bf16, tag=f"x0_{kq}", name=f"x0_{kq}")
                eng.dma_start(xt0[:], xt_e[kq * P : (kq + 1) * P, 0:CH])
                x0.append(xt0)
                bt = bpool.tile([P, H], bf16, tag=f"b_{kq}", name=f"b_{kq}")
                eng.dma_start(bt[:], b_e[kq * P : (kq + 1) * P, :])
                b_sb.append(bt)
            for kq in range(KQ):
                eng = nc.sync if kq % 2 == 0 else nc.scalar
                xt12 = xpool.tile(
                    [P, W - CH], bf16, tag=f"x12_{kq}", name=f"x12_{kq}"
                )
                eng.dma_start(xt12[:], xt_e[kq * P : (kq + 1) * P, CH:W])
                x12.append(xt12)

            h_t = [
                hpool.tile([P, W], bf16, tag=f"h{g}", name=f"h{g}")
                for g in range(G)
            ]

            def scan_and_store(g, ni, ps_g):
                n0 = ni * CH
                init = 0.0 if ni == 0 else h_t[g][:, n0 - 1 : n0]
                nc.vector.tensor_tensor_scan(
                    h_t[g][:, n0 : n0 + CH],
                    a_op[g],
                    ps_g[:],
                    init,
                    op0=mybir.AluOpType.mult,
                    op1=mybir.AluOpType.add,
                )
                lo = HALO if ni == 0 else 0
                nc.scalar.dma_start(
                    out_e[g * P : (g + 1) * P, n0 + lo - HALO : n0 + CH - HALO],
                    h_t[g][:, n0 + lo : n0 + CH],
                )

            # chunk 0: kq-major across all 8 PSUM banks (DMA-arrival matched)
            ps0 = [
                pspool.tile([P, CH], f32, tag="ps", name=f"ps0_{g}")
                for g in range(G)
            ]
            for kq in range(KQ):
                for g in range(G):
                    nc.tensor.matmul(
                        ps0[g][:],
                        b_sb[kq][:, g * P : (g + 1) * P],
                        x0[kq][:],
                        start=(kq == 0),
                        stop=(kq == KQ - 1),
                    )
            for g in range(G):
                scan_and_store(g, 0, ps0[g])

            # chunks 1-2: g-major, scans chase and free banks just in time
            for ni in (1, 2):
                for g in range(G):
                    ps_g = pspool.tile([P, CH], f32, tag="ps", name=f"ps{ni}_{g}")
                    for kq in range(KQ):
                        nc.tensor.matmul(
                            ps_g[:],
                            b_sb[kq][:, g * P : (g + 1) * P],
                            x12[kq][:, (ni - 1) * CH : ni * CH],
                            start=(kq == 0),
                            stop=(kq == KQ - 1),
                        )
                    scan_and_store(g, ni, ps_g)

    nc.compile()
    return nc


def _get_nc():
    if "nc" not in _state:
        _state["nc"] = _build_nc()
    return _state["nc"]


def _shard_inputs(x_seq, a_diag, b_mat):
    x = np.asarray(x_seq, dtype=np.float32)
    a = np.asarray(a_diag, dtype=np.float32)
    b = np.asarray(b_mat, dtype=np.float32)
    bq = np.ascontiguousarray(b.astype(ml_dtypes.bfloat16))
    x_pad = np.concatenate([np.zeros((HALO, H), np.float32), x], axis=0)
    xT = np.ascontiguousarray(x_pad.T).astype(ml_dtypes.bfloat16)  # [H, T+HALO]
    av = np.ascontiguousarray(a.reshape(G, P).T)  # [P, G]
    in_maps = []
    for i in range(NC):
        in_maps.append(
            {
                "xt": np.ascontiguousarray(xT[:, i * T_LOC : i * T_LOC + W]),
                "b": bq,
                "av": av,
            }
        )
    return in_maps


def kernel(x_seq, a_diag, b_mat):
    from concourse.bass_utils import run_bass_kernel_spmd

    nc = _get_nc()
    in_maps = _shard_inputs(x_seq, a_diag, b_mat)
    res = run_bass_kernel_spmd(nc, in_maps, list(range(NC)))
    _state["last_result"] = res
    out = np.concatenate(
        [
            np.asarray(res.results[i]["out"]).astype(np.float32).T
            for i in range(NC)
        ],
        axis=0,
    )
    return out


# revision 6
# speedup vs baseline: 1.2784x; 1.0486x over previous
"""DiagSSMBlock Trainium2 kernel.

h_t = sum_{k=0..t} a^k * (B^T x_{t-k})  ==  h_t = a * h_{t-1} + s_t, s = B^T x^T.

Strategy: shard T across the 8 cores (1024 steps each + 8-step halo; |a| <=
sqrt(2/1024) ~ 0.044 so a^8 ~ 1.5e-11 — invisible at fp32, making slabs
independent).  Host passes x pre-transposed ([H, T_slab]) and pre-cast to
bf16 along with B; bf16 keeps the PE at 1 cycle/row (same rate as fp32r)
while halving HBM traffic.

Layout decisions driven by measured ntff traces:
  - ~7 us framework preamble before any engine does work; warm-up matmuls
    bridge from preamble end to first-data arrival so the PE HAM clock-gate
    is at 2.4 GHz when real matmuls start (N_WARM=36 tuned to that window);
  - each dma_start costs ~0.6 us of issue + ~0.8 us of per-ring completion
    serialization, so loads are batched (b in 2-row pairs, x chunk-1/2 and
    x0-rest as single multi-segment DMAs) and split across BOTH HWDGE rings
    (sync + scalar) in consumption order; no SWDGE (gpsimd) DMAs at all —
    its descriptor rings contend with the HWDGE AXI ports;
  - chunk ni=0 runs kq-MAJOR across all 8 PSUM banks (one per 128-channel
    group) so compute starts when (x0[0], b0) ~350 KB have landed, and each
    pass consumes one b row-pair-half as it arrives;
  - chunks 1-2 run g-major: the DVE scan of group g frees its PSUM bank
    just ahead of the next chunk's reuse — no 8-scan serial barrier;
  - the SSM recurrence is DVE tensor_tensor_scan (1x mode only — the ISA
    has no 2x uop for scans), reading PSUM directly, per-partition `a` via
    stride-0 broadcast AP, bf16 output written straight into the h tile;
  - stores alternate between the two rings in scan-completion order.
"""

import sys

if "/opt/trn_rl_repo" not in sys.path:
    sys.path.insert(0, "/opt/trn_rl_repo")

import ml_dtypes
import numpy as np

T, H = 8192, 1024
NC = 8
P = 128
T_LOC = T // NC            # 1024 output timesteps per core
HALO = 8                   # scan warmup; a^8 ~ 1.5e-11
W = T_LOC + HALO           # 1032
CH = 344                   # psum chunk width (3 chunks of 344 = 1032)
NCHUNK = W // CH           # 3
KQ = H // P                # 8 contraction chunks
G = H // P                 # 8 output-channel groups
N_WARM = 36                # dummy matmuls to lift the HAM clock gate

_state = {}


def _build_nc():
    import concourse.tile as tile
    from concourse import bacc, mybir

    bf16 = mybir.dt.bfloat16
    f32 = mybir.dt.float32

    nc = bacc.Bacc("TRN2", target_bir_lowering=False, debug=False, num_devices=NC)
    xt_e = nc.dram_tensor("xt", [H, W], bf16, kind="ExternalInput").ap()
    b_e = nc.dram_tensor("b", [H, H], bf16, kind="ExternalInput").ap()
    av_e = nc.dram_tensor("av", [P, G], f32, kind="ExternalInput").ap()
    out_e = nc.dram_tensor("out", [H, T_LOC], bf16, kind="ExternalOutput").ap()
    flush_e = nc.dram_tensor("warm_flush", [P, 1], f32).ap()

    # kq-indexed views: row (q*128 + p) -> [p, q, col]
    xt_r = xt_e.rearrange("(q p) w -> p q w", p=P)
    b_r = b_e.rearrange("(q p) c -> p q c", p=P)

    with tile.TileContext(nc) as tc:
        with (
            tc.tile_pool(name="consts", bufs=1) as consts,
            tc.tile_pool(name="bpool", bufs=1) as bpool,
            tc.tile_pool(name="xpool", bufs=1) as xpool,
            tc.tile_pool(name="hpool", bufs=1) as hpool,
            tc.tile_pool(name="pspool", bufs=8, space="PSUM") as pspool,
        ):
            # PE warm-up: bridge preamble-end -> first-data with dummy MMs.
            warm_sb = consts.tile([P, P], bf16, tag="warm")
            nc.gpsimd.memset(warm_sb[:], 0.0)
            wps = pspool.tile([P, P], f32, tag="ps", name="wps")
            for i in range(N_WARM):
                nc.tensor.matmul(
                    wps[:],
                    warm_sb[:],
                    warm_sb[:],
                    start=(i == 0),
                    stop=(i == N_WARM - 1),
                )
            flush_sb = consts.tile([P, 1], f32, tag="flush")
            nc.vector.tensor_copy(flush_sb[:], wps[:, 0:1])

            # a values: tiny HWDGE load, first on sync
            av_sb = consts.tile([P, G], f32, tag="av")
            nc.sync.dma_start(av_sb[:], av_e[:])
            a_op = [av_sb[:, g : g + 1].broadcast_to([P, CH]) for g in range(G)]

            # ---- input loads: batched, both HWDGE rings, consumption order
            # sync:   av, x0[0], b0, b23, b67, x12_ev
            # scalar: x0_rest,  b1, b45,      x12_od
            x0_first = xpool.tile([P, CH], bf16, tag="x0f", name="x0_first")
            nc.sync.dma_start(x0_first[:], xt_e[0:P, 0:CH])
            b0 = bpool.tile([P, H], bf16, tag="b0", name="b0")
            nc.sync.dma_start(b0[:], b_e[0:P, :])

            x0_rest = xpool.tile([P, (KQ - 1) * CH], bf16, tag="x0r", name="x0_rest")
            nc.scalar.dma_start(x0_rest[:], xt_r[:, 1:KQ, 0:CH])
            b1 = bpool.tile([P, H], bf16, tag="b1", name="b1")
            nc.scalar.dma_start(b1[:], b_e[P : 2 * P, :])

            b23 = bpool.tile([P, 2 * H], bf16, tag="b23", name="b23")
            nc.sync.dma_start(b23[:], b_r[:, 2:4, :])
            b45 = bpool.tile([P, 2 * H], bf16, tag="b45", name="b45")
            nc.scalar.dma_start(b45[:], b_r[:, 4:6, :])
            b67 = bpool.tile([P, 2 * H], bf16, tag="b67", name="b67")
            nc.sync.dma_start(b67[:], b_r[:, 6:8, :])

            x12_ev = xpool.tile([P, 4 * (W - CH)], bf16, tag="x12e", name="x12_ev")
            nc.sync.dma_start(x12_ev[:], xt_r[:, 0:KQ:2, CH:W])
            x12_od = xpool.tile([P, 4 * (W - CH)], bf16, tag="x12o", name="x12_od")
            nc.scalar.dma_start(x12_od[:], xt_r[:, 1:KQ:2, CH:W])

            def b_slice(kq, g):
                if kq == 0:
                    return b0[:, g * P : (g + 1) * P]
                if kq == 1:
                    return b1[:, g * P : (g + 1) * P]
                pair, off = divmod(kq - 2, 2)
                t = (b23, b45, b67)[pair]
                return t[:, off * H + g * P : off * H + (g + 1) * P]

            def x_chunk(kq, ni):
                if ni == 0:
                    if kq == 0:
                        return x0_first[:]
                    return x0_rest[:, (kq - 1) * CH : kq * CH]
                t = x12_ev if kq % 2 == 0 else x12_od
                base = (kq // 2) * (W - CH) + (ni - 1) * CH
                return t[:, base : base + CH]

            h_t = [
                hpool.tile([P, W], bf16, tag=f"h{g}", name=f"h{g}")
                for g in range(G)
            ]

            def scan_and_store(g, ni, ps_g):
                n0 = ni * CH
                init = 0.0 if ni == 0 else h_t[g][:, n0 - 1 : n0]
                nc.vector.tensor_tensor_scan(
                    h_t[g][:, n0 : n0 + CH],
                    a_op[g],
                    ps_g[:],
                    init,
                    op0=mybir.AluOpType.mult,
                    op1=mybir.AluOpType.add,
                )
                lo = HALO if ni == 0 else 0
                eng = nc.scalar if g % 2 == 0 else nc.sync
                eng.dma_start(
                    out_e[g * P : (g + 1) * P, n0 + lo - HALO : n0 + CH - HALO],
                    h_t[g][:, n0 + lo : n0 + CH],
                )

            # chunk 0: kq-major across all 8 PSUM banks (DMA-arrival matched)
            ps0 = [
                pspool.tile([P, CH], f32, tag="ps", name=f"ps0_{g}")
                for g in range(G)
            ]
            for kq in range(KQ):
                for g in range(G):
                    nc.tensor.matmul(
                        ps0[g][:],
                        b_slice(kq, g),
                        x_chunk(kq, 0),
                        start=(kq == 0),
                        stop=(kq == KQ - 1),
                    )
            for g in range(G):
                scan_and_store(g, 0, ps0[g])

            # chunks 1-2: g-major, scans chase and free banks just in time
            for ni in (1, 2):
                for g in range(G):
                    ps_g = pspool.tile([P, CH], f32, tag="ps", name=f"ps{ni}_{g}")
                    for kq in range(KQ):
                        nc.tensor.matmul(
                            ps_g[:],
                            b_slice(kq, g),
                            x_chunk(kq, ni),
                            start=(kq == 0),
                            stop=(kq == KQ - 1),
                        )
                    scan_and_store(g, ni, ps_g)

            # warm-MM flush store, late, on scalar's ring (anti-DCE)
            nc.scalar.dma_start(flush_e[:], flush_sb[:])

    nc.compile()
    return nc


def _get_nc():
    if "nc" not in _state:
        _state["nc"] = _build_nc()
    return _state["nc"]


def _shard_inputs(x_seq, a_diag, b_mat):
    x = np.asarray(x_seq, dtype=np.float32)
    a = np.asarray(a_diag, dtype=np.float32)
    b = np.asarray(b_mat, dtype=np.float32)
    bq = np.ascontiguousarray(b.astype(ml_dtypes.bfloat16))
    x_pad = np.concatenate([np.zeros((HALO, H), np.float32), x], axis=0)
    xT = np.ascontiguousarray(x_pad.T).astype(ml_dtypes.bfloat16)  # [H, T+HALO]
    av = np.ascontiguousarray(a.reshape(G, P).T)  # [P, G]
    in_maps = []
    for i in range(NC):
        in_maps.append(
            {
                "xt": np.ascontiguousarray(xT[:, i * T_LOC : i * T_LOC + W]),
                "b": bq,
                "av": av,
            }
        )
    return in_maps


def kernel(x_seq, a_diag, b_mat):
    from concourse.bass_utils import run_bass_kernel_spmd

    nc = _get_nc()
    in_maps = _shard_inputs(x_seq, a_diag, b_mat)
    res = run_bass_kernel_spmd(nc, in_maps, list(range(NC)))
    _state["last_result"] = res
    out = np.concatenate(
        [
            np.asarray(res.results[i]["out"]).astype(np.float32).T
            for i in range(NC)
        ],
        axis=0,
    )
    return out
